# revision 1
# baseline (speedup 1.0000x reference)
"""Trainium2 Bass kernel for the MANE multi-view SGNS embedding loss.

Strategy: data-parallel over the batch axis B across 8 NeuronCores with the
embedding tables replicated per core (each core sees the two tables
concatenated into one [6*N, D] DRAM tensor).  All embedding-row gathers run
as SWDGE indirect DMAs; dot products run as fused tensor_tensor_reduce ops
on the vector engine; log-sigmoid + per-term reduction runs on the scalar
engine via Softplus with accum_out.  Per-core partial sums [128, 15] are
combined on the host (scalar all-reduce).
"""

import numpy as np

import concourse.bass as bass
import concourse.bacc as bacc
import concourse.tile as tile
from concourse import mybir
from concourse.bass_utils import run_bass_kernel_spmd

# ---------------------------------------------------------------- problem dims
V, N, D = 3, 200000, 128
B, K = 32768, 10
TOTAL = 65536
NCORES = 8
P = 128
T = 3 + 2 * V * (V - 1)  # 15 terms

F32 = mybir.dt.float32
I32 = mybir.dt.int32

# (j, i) pairs in reference order for cost2/cost3
PAIRS = [(j, i) for j in range(V) for i in range(V) if i != j]
# center view per term: cost1[i] -> i, cost2/3 (j,i) -> i
TERM_VIEW = [0, 1, 2] + [i for (_, i) in PAIRS] + [i for (_, i) in PAIRS]


def build_bass(bc, k, nchunk, n_rows=2 * V * N):
    """Build + compile the per-core Tile program.

    bc: batch elems per core; k: negatives per positive; nchunk: number of
    gather/compute chunks (each chunk covers bc//nchunk batch elems).
    """
    chunk = bc // nchunk
    jb = chunk // P           # batch elems per partition per chunk
    assert jb * P * nchunk == bc

    nc = bacc.Bacc("TRN2", target_bir_lowering=False, debug=False,
                   enable_asserts=False)

    W = nc.dram_tensor("w_all", [n_rows, D], F32, kind="ExternalInput")
    cidx = nc.dram_tensor("cidx", [V, P, nchunk * jb], I32, kind="ExternalInput")
    pidx = nc.dram_tensor("pidx", [T, P, nchunk * jb], I32, kind="ExternalInput")
    nidx = nc.dram_tensor("nidx", [T, P, nchunk * jb * k], I32, kind="ExternalInput")
    acc_out = nc.dram_tensor("acc", [P, 2 * T], F32, kind="ExternalOutput")

    from contextlib import ExitStack
    with tile.TileContext(nc) as tc, ExitStack() as ctx:
        cen_pool = ctx.enter_context(tc.tile_pool(name="cen", bufs=1))
        idx_pool = ctx.enter_context(tc.tile_pool(name="idx", bufs=2))
        gat_pool = ctx.enter_context(tc.tile_pool(name="gat", bufs=8))
        x_pool = ctx.enter_context(tc.tile_pool(name="x", bufs=2))
        scr_pool = ctx.enter_context(tc.tile_pool(name="scr", bufs=3))
        out_pool = ctx.enter_context(tc.tile_pool(name="out", bufs=1))

        # ---- centers: gather node embeddings for each view, chunk-local layout
        # (indirect_dma_start gathers exactly one row per partition per call:
        # out [P, D] with a [P, 1] index column.)
        CEN = []  # CEN[v][c]: [P, jb*D]
        for v in range(V):
            cit = idx_pool.tile([P, nchunk * jb], I32, tag="cidx")
            nc.sync.dma_start(cit[:], cidx.ap()[v])
            tiles_v = []
            for c in range(nchunk):
                ct = cen_pool.tile([P, jb * D], F32, tag=f"cen_{v}_{c}")
                for j in range(jb):
                    cc = c * jb + j
                    nc.gpsimd.indirect_dma_start(
                        out=ct[:, j * D:(j + 1) * D], out_offset=None,
                        in_=W.ap(),
                        in_offset=bass.IndirectOffsetOnAxis(
                            ap=cit[:, cc:cc + 1], axis=0),
                    )
                tiles_v.append(ct)
            CEN.append(tiles_v)

        # ACC columns: [0:T] = neg sums, [T:2T] = pos sums
        ACC = out_pool.tile([P, 2 * T], F32)

        for t in range(T):
            iv = TERM_VIEW[t]
            pit = idx_pool.tile([P, nchunk * jb], I32, tag="pidx")
            nc.sync.dma_start(pit[:], pidx.ap()[t])
            nit = idx_pool.tile([P, nchunk * jb * k], I32, tag="nidx")
            nc.sync.dma_start(nit[:], nidx.ap()[t])

            XN = x_pool.tile([P, nchunk * jb * k], F32, tag="XN")
            XP = x_pool.tile([P, nchunk * jb], F32, tag="XP")
            for c in range(nchunk):
                for j in range(jb):
                    cj = c * jb + j
                    cen_ap = CEN[iv][c][:, j * D:(j + 1) * D]
                    NEGJ = gat_pool.tile([P, k * D], F32, tag="negj")
                    for kk in range(k):
                        ncol = c * jb * k + j * k + kk
                        nc.gpsimd.indirect_dma_start(
                            out=NEGJ[:, kk * D:(kk + 1) * D], out_offset=None,
                            in_=W.ap(),
                            in_offset=bass.IndirectOffsetOnAxis(
                                ap=nit[:, ncol:ncol + 1], axis=0),
                        )
                    PG = gat_pool.tile([P, D], F32, tag="pg")
                    nc.gpsimd.indirect_dma_start(
                        out=PG[:], out_offset=None,
                        in_=W.ap(),
                        in_offset=bass.IndirectOffsetOnAxis(
                            ap=pit[:, cj:cj + 1], axis=0),
                    )
                    prod = scr_pool.tile([P, k * D], F32, tag="prod")
                    nc.vector.tensor_tensor(
                        out=prod[:].rearrange("p (k d) -> p k d", k=k),
                        in0=NEGJ[:].rearrange("p (k d) -> p k d", k=k),
                        in1=cen_ap.unsqueeze(1).to_broadcast([P, k, D]),
                        op=mybir.AluOpType.mult)
                    nc.vector.tensor_reduce(
                        out=XN[:, cj * k:(cj + 1) * k],
                        in_=prod[:].rearrange("p (k d) -> p k d", k=k),
                        axis=mybir.AxisListType.X, op=mybir.AluOpType.add)
                    prodp = scr_pool.tile([P, D], F32, tag="prodp")
                    nc.vector.tensor_tensor(
                        out=prodp[:], in0=PG[:], in1=cen_ap,
                        op=mybir.AluOpType.mult)
                    nc.vector.tensor_reduce(
                        out=XP[:, cj:cj + 1],
                        in_=prodp[:].unsqueeze(1),
                        axis=mybir.AxisListType.X, op=mybir.AluOpType.add)
            # log_sigmoid: neg sum uses sigmoid(-x), pos sum uses sigmoid(x)
            sgn = scr_pool.tile([P, nchunk * jb * k], F32, tag="sgn")
            nc.scalar.activation(
                out=sgn[:], in_=XN[:],
                func=mybir.ActivationFunctionType.Sigmoid, scale=-1.0)
            spn = scr_pool.tile([P, nchunk * jb * k], F32, tag="spn")
            nc.scalar.activation(
                out=spn[:], in_=sgn[:],
                func=mybir.ActivationFunctionType.Ln,
                accum_out=ACC[:, t:t + 1])
            sgp = scr_pool.tile([P, nchunk * jb], F32, tag="sgp")
            nc.scalar.activation(
                out=sgp[:], in_=XP[:],
                func=mybir.ActivationFunctionType.Sigmoid)
            spp = scr_pool.tile([P, nchunk * jb], F32, tag="spp")
            nc.scalar.activation(
                out=spp[:], in_=sgp[:],
                func=mybir.ActivationFunctionType.Ln,
                accum_out=ACC[:, T + t:T + t + 1])

        nc.sync.dma_start(acc_out.ap(), ACC[:])

    nc.compile()
    return nc


_NC_CACHE = {}


def _get_nc(bc, k, nchunk):
    key = (bc, k, nchunk)
    if key not in _NC_CACHE:
        _NC_CACHE[key] = build_bass(bc, k, nchunk)
    return _NC_CACHE[key]


def _lay2(x, nchunk, jb):
    # x: [..., bc] -> [..., P, nchunk*jb] with [c,p,j] -> col c*jb+j on partition p
    lead = x.shape[:-1]
    return (x.reshape(*lead, nchunk, P, jb)
             .swapaxes(-3, -2)
             .reshape(*lead, P, nchunk * jb))


def _lay3(x, nchunk, jb, k):
    # x: [..., bc, k] -> [..., P, nchunk*jb*k]
    lead = x.shape[:-2]
    return (x.reshape(*lead, nchunk, P, jb, k)
             .swapaxes(-4, -3)
             .reshape(*lead, P, nchunk * jb * k))


def host_prep(count, shuffle_indices, nodes_idx, neigh_idx,
              neg_idx1, neg_idx2, neg_idx3, node_W, neigh_W,
              n_cores=NCORES, nchunk=4, b=B):
    """Compute per-core input maps + the W table. Pure numpy."""
    c0 = int(count)
    sh = np.asarray(shuffle_indices)[:, c0:c0 + b].astype(np.int64)
    nodes_sel = np.take_along_axis(np.asarray(nodes_idx).astype(np.int64), sh, axis=1)
    neigh_sel = np.take_along_axis(np.asarray(neigh_idx).astype(np.int64), sh, axis=1)
    neg1 = np.asarray(neg_idx1).astype(np.int64)[:, :b]
    neg2 = np.asarray(neg_idx2).astype(np.int64)[:, :, :b]
    neg3 = np.asarray(neg_idx3).astype(np.int64)[:, :, :b]

    node_W = np.ascontiguousarray(np.asarray(node_W), dtype=np.float32)
    neigh_W = np.ascontiguousarray(np.asarray(neigh_W), dtype=np.float32)
    n = node_W.shape[1]
    d = node_W.shape[2]
    W_all = np.concatenate(
        [node_W.reshape(V * n, d), neigh_W.reshape(V * n, d)], axis=0)

    # per-term (pos_idx [V? ...], neg_idx, view) with global row offsets
    pos_list, neg_list = [], []
    for i in range(V):
        pos_list.append(neigh_sel[i] + (V + i) * n)
        neg_list.append(neg1[i] + (V + i) * n)
    for (j, i) in PAIRS:
        pos_list.append(nodes_sel[i] + j * n)
        neg_list.append(neg2[j, i] + j * n)
    for (j, i) in PAIRS:
        pos_list.append(neigh_sel[i] + (V + j) * n)
        neg_list.append(neg3[j, i] + (V + j) * n)
    pos_all = np.stack(pos_list)          # [T, b]
    neg_all = np.stack(neg_list)          # [T, b, K]
    cen_all = nodes_sel + (np.arange(V) * n)[:, None]  # [T? V, b]

    bc = b // n_cores
    chunk = bc // nchunk
    jb = chunk // P
    k = neg_all.shape[-1]

    in_maps = []
    for core in range(n_cores):
        sl = slice(core * bc, (core + 1) * bc)
        in_maps.append({
            "w_all": W_all,
            "cidx": _lay2(cen_all[:, sl], nchunk, jb).astype(np.int32),
            "pidx": _lay2(pos_all[:, sl], nchunk, jb).astype(np.int32),
            "nidx": _lay3(neg_all[:, sl], nchunk, jb, k).astype(np.int32),
        })
    return in_maps


def host_combine(acc_list, hyp1, hyp2, b=B):
    """acc_list: per-core [P, T] softplus-sum partials -> final scalar."""
    s = np.zeros(T, dtype=np.float64)
    for a in acc_list:
        a = np.asarray(a, dtype=np.float64).sum(axis=0)
        s += a[:T] + a[T:2 * T]
    term_val = s / b
    cost1 = term_val[0:3].mean()
    cost2 = float(np.asarray(hyp1).reshape(-1)[0]) * term_val[3:9].sum() / 6.0
    cost3 = float(np.asarray(hyp2).reshape(-1)[0]) * term_val[9:15].sum() / 6.0
    return np.array(-(cost1 + cost2 + cost3) / 3.0, dtype=np.float32)


def kernel(count, shuffle_indices, nodes_idx, neigh_idx,
           neg_idx1, neg_idx2, neg_idx3, node_W, neigh_W, hyp1, hyp2):
    in_maps = host_prep(count, shuffle_indices, nodes_idx, neigh_idx,
                        neg_idx1, neg_idx2, neg_idx3, node_W, neigh_W)
    nc = _get_nc(B // NCORES, K, 4)
    res = run_bass_kernel_spmd(nc, in_maps, core_ids=list(range(NCORES)))
    acc_list = [r["acc"] for r in res.results]
    return host_combine(acc_list, hyp1, hyp2)

